# revision 35
# baseline (speedup 1.0000x reference)
"""Distributed multi-head attention kernel for one TRN2 chip (8 NeuronCores).

Problem: B=2, T=2048, D=1024, H=16 heads (hd=64).
  qkv = x @ w_attn + b_attn ; per-head softmax((q k^T)/sqrt(hd) + 2*mask) v
  out = attn @ w_proj + b_proj

Sharding: tensor-parallel over heads. Core c owns heads {2c, 2c+1}.
  - QKV projection computed in transposed layout (QT/KT: [hd, T]) from
    host-transposed XT, packed so every matmul uses the full 128-wide PE.
  - Attention runs in transposed space: S^T[kv, q] = KT^T-block @ QT-block,
    so the additive mask is a per-partition bias folded into the Exp, and
    the softmax denominator falls out of a ones-column appended to V in the
    PV matmul (O_ext = [V|1]^T @ P^T).
  - Mid-kernel AllToAll (two halves, one per batch) reshards from
    head-space to row-space; the final projection then needs no reduction.
  - All matmuls use float32r (full-rate fp32 PE mode, moving dim 512).
"""

import sys

sys.path.insert(0, "/opt/trn_rl_repo")

import numpy as np

B, T, D = 2, 2048, 1024
H = 16
HD = D // H
NCORES = 8
HPC = H // NCORES          # heads per core = 2
BT = B * T                 # 4096 global rows
ROWS_PER_CORE = BT // NCORES   # 512
RHALF = ROWS_PER_CORE // 2     # 256 rows per A2A half
TB = 512                   # T-block width for QKV projection
NTB = BT // TB             # 8
NKD = D // 128             # 8 contraction chunks over D
QB = 512                   # q-block width in attention
NQB = T // QB              # 4 per (batch, head)
NKV = T // 128             # 16 kv chunks per batch

_CACHE = {}
import ml_dtypes
BF16 = ml_dtypes.bfloat16
_ZEROS = np.zeros((64, 2048), BF16)


def _build(with_battn: bool, with_bproj: bool):
    import concourse.bass as bass
    import concourse.tile as tile
    from concourse import bacc, mybir
    from concourse.masks import make_identity

    f32 = mybir.dt.float32
    bf16 = mybir.dt.bfloat16
    Exp = mybir.ActivationFunctionType.Exp

    nc = bacc.Bacc("TRN2", target_bir_lowering=False, debug=False,
                   num_devices=NCORES)
    rg = [list(range(NCORES))]

    xt = nc.dram_tensor("xt", [D, BT], bf16, kind="ExternalInput")
    w_qkv = nc.dram_tensor("w_qkv", [D, 3 * 128], bf16, kind="ExternalInput")
    w_proj = nc.dram_tensor("w_proj", [D, D], bf16, kind="ExternalInput")
    mask2 = nc.dram_tensor("mask2", [128, B * (T // 128)], f32,
                           kind="ExternalInput")
    zeros_in = nc.dram_tensor("zeros_in", [64, 2048], bf16, kind="ExternalInput")
    if with_battn:
        b_qkv = nc.dram_tensor("b_qkv", [1, 3 * 128], bf16, kind="ExternalInput")
    if with_bproj:
        b_proj = nc.dram_tensor("b_proj", [1, D], bf16, kind="ExternalInput")
    out = nc.dram_tensor("out", [ROWS_PER_CORE, D], f32, kind="ExternalOutput")

    with tile.TileContext(nc, num_cores=NCORES) as tc:
        from contextlib import ExitStack
        with ExitStack() as ctx:
            const = ctx.enter_context(tc.tile_pool(name="const", bufs=1))
            xt_pool = ctx.enter_context(tc.tile_pool(name="xtp", bufs=10))
            qk_pool = ctx.enter_context(tc.tile_pool(name="qkp", bufs=1))
            vnat_pool = ctx.enter_context(tc.tile_pool(name="vnat", bufs=64))
            pt_pool = ctx.enter_context(tc.tile_pool(name="ptp", bufs=5))
            lbc_pool = ctx.enter_context(tc.tile_pool(name="lbc", bufs=2))
            pin_pool = ctx.enter_context(tc.tile_pool(name="pin", bufs=8))
            out_pool = ctx.enter_context(tc.tile_pool(name="outp", bufs=2))
            ps_mm = ctx.enter_context(tc.tile_pool(name="psmm", bufs=4, space="PSUM"))
            ps_ot = ctx.enter_context(tc.tile_pool(name="psot", bufs=4, space="PSUM"))
            dram = ctx.enter_context(tc.tile_pool(name="dram", bufs=2, space="DRAM"))

            # ---- constants ----
            # w_qkv packed per D-chunk: [128, NKD, 384]; group g cols
            # [128g, 128g+128): g0=[q_h0/8|k_h0] g1=[q_h1/8|k_h1] g2=[v_h0|v_h1]
            wqkv_sb = const.tile([128, NKD, 3 * 128], bf16)
            for d in range(NKD):
                nc.sync.dma_start(out=wqkv_sb[:, d, :],
                                  in_=w_qkv[128 * d:128 * (d + 1), :])
            wproj_sb = const.tile([128, NKD, D], bf16)

            def emit_wproj_loads():
                for j in range(NKD):
                    nc.sync.dma_start(out=wproj_sb[:, j, :],
                                      in_=w_proj[128 * j:128 * (j + 1), :])
            # mask (already doubled on host): [128, B, NKV]
            mask_sb = const.tile([128, B, NKV], f32)
            nc.sync.dma_start(out=mask_sb[:],
                              in_=mask2[:].rearrange("p (b j) -> p b j", b=B))
            ident = const.tile([128, 128], bf16)
            make_identity(nc, ident[:])

            if with_battn:
                bqkv_sb = const.tile([1, 3 * 128], bf16)
                nc.sync.dma_start(out=bqkv_sb[:], in_=b_qkv[:])
                ones_row = const.tile([1, TB], bf16)
                nc.vector.memset(ones_row[:], 1.0)
            if with_bproj:
                bproj_sb = const.tile([1, D], bf16)
                nc.sync.dma_start(out=bproj_sb[:], in_=b_proj[:])
                ones_col = const.tile([1, 128], bf16)
                nc.vector.memset(ones_col[:], 1.0)

            # persistent activations. Head h lives at partitions [64h, 64h+64)
            # of both qt2 and kt2, so S^T matmul operands share a base.
            qt2 = qk_pool.tile([128, BT], bf16, tag="qt2", name="qt2")
            # K for head h zero-padded to 128 rows so the S^T matmul streams
            # full-width operands: ktA = [K_h0; 0], ktB = [0; K_h1]
            ktA = qk_pool.tile([128, BT], bf16, tag="ktA", name="ktA")
            ktB = qk_pool.tile([128, BT], bf16, tag="ktB", name="ktB")
            for half in range(2):
                cs = 2048 * half
                nc.sync.dma_start(out=ktA[64:128, cs:cs + 2048],
                                  in_=zeros_in[:])
                nc.sync.dma_start(out=ktB[0:64, cs:cs + 2048],
                                  in_=zeros_in[:])
            vt2 = qk_pool.tile([128, BT], bf16, tag="vt2", name="vt2")
            ot = qk_pool.tile([128, BT], bf16, tag="ot", name="ot")

            # ---- QKV projection (transposed outputs) ----
            def qkv_tblock(tb):
                xts = []
                for d in range(NKD):
                    t = xt_pool.tile([128, TB], bf16, tag="xt", name="xt")
                    nc.sync.dma_start(
                        out=t[:], in_=xt[128 * d:128 * (d + 1),
                                         TB * tb:TB * (tb + 1)])
                    xts.append(t)
                for g in range(3):
                    ps = ps_mm.tile([128, TB], f32, tag="mm", name="mm")
                    for d in range(NKD):
                        nc.tensor.matmul(
                            ps[:],
                            lhsT=wqkv_sb[:, d, 128 * g:128 * (g + 1)],
                            rhs=xts[d][:],
                            start=(d == 0),
                            stop=(d == NKD - 1) and not with_battn)
                    if with_battn:
                        nc.tensor.matmul(
                            ps[:],
                            lhsT=bqkv_sb[:, 128 * g:128 * (g + 1)],
                            rhs=ones_row[:],
                            start=False, stop=True)
                    if g < 2:
                        nc.scalar.copy(
                            out=qt2[64 * g:64 * (g + 1), TB * tb:TB * (tb + 1)],
                            in_=ps[0:64, :])
                        ktdst = ktA if g == 0 else ktB
                        nc.scalar.copy(
                            out=ktdst[64 * g:64 * (g + 1), TB * tb:TB * (tb + 1)],
                            in_=ps[64:128, :])
                    else:
                        nc.vector.tensor_copy(
                            out=vt2[:, TB * tb:TB * (tb + 1)], in_=ps[:])

            # ---- V transpose to natural layout (+ ones col for denom) ----
            vnat = {}

            def v_transpose(b, js=None):
                for j in (range(NKV) if js is None else js):
                    pst = ps_mm.tile([128, 128], bf16, tag="mm", name="tr",
                                     padded_shape=[128, 512])
                    nc.tensor.transpose(
                        pst[:], vt2[:, 2048 * b + 128 * j: 2048 * b + 128 * (j + 1)],
                        ident[:])
                    for h in range(HPC):
                        vtile = vnat_pool.tile([128, 128], bf16, tag="vnat", name="vnat")
                        nc.vector.tensor_copy(out=vtile[:, 0:64],
                                              in_=pst[:, 64 * h:64 * (h + 1)])
                        nc.vector.memset(vtile[:, 64:65], 1.0)
                        vnat[(b, h, j)] = vtile

            # ---- attention for one (batch, q-block), both heads ----
            # Head h operands live at partition base 64h, so the two S^T
            # matmuls land on disjoint PE row-groups (tile_position auto
            # (0,0)/(64,0)) and execute concurrently.
            def attn_qblock(b, qb):
                c0 = 2048 * b + QB * qb
                qsl = qt2[:, c0:c0 + QB]
                ps_o = [ps_ot.tile([128, QB], f32, tag="ot", name="ot")
                        for _ in range(HPC)]
                for j in range(NKV):
                    k0 = 2048 * b + 128 * j
                    pts = []
                    for h in range(HPC):
                        ps_s = ps_mm.tile([128, QB], f32, tag="mm", name="mm")
                        nc.tensor.matmul(
                            ps_s[:],
                            lhsT=(ktA if h == 0 else ktB)[:, k0:k0 + 128],
                            rhs=qsl,
                            start=True, stop=True)
                        pt = pt_pool.tile([128, QB], bf16, tag="pt", name="pt")
                        nc.scalar.activation(out=pt[:], in_=ps_s[:], func=Exp,
                                             bias=mask_sb[:, b, j:j + 1],
                                             scale=1.0)
                        pts.append(pt)
                    for h in range(HPC):
                        nc.tensor.matmul(
                            ps_o[h][:],
                            lhsT=vnat[(b, h, j)][:],  # [V_h|1|pad] full width
                            rhs=pts[h][:],
                            start=(j == 0), stop=(j == NKV - 1),
                            skip_group_check=True)
                # normalize by softmax denom (row 64 of ps_o) and write OT
                for h in range(HPC):
                    lsb = lbc_pool.tile([1, QB], f32, tag="lsb", name="lsb")
                    nc.vector.tensor_copy(out=lsb[:], in_=ps_o[h][64:65, :])
                    ldram = dram.tile([1, QB], f32, tag="ld", name="ld")
                    nc.sync.dma_start(out=ldram[:], in_=lsb[:])
                    lbc = lbc_pool.tile([64, QB], f32, tag="lbc", name="lbc")
                    nc.sync.dma_start(out=lbc[:],
                                      in_=ldram[:].to_broadcast([64, QB]))
                    lrec = lbc_pool.tile([64, QB], f32, tag="lrec", name="lrec")
                    nc.vector.reciprocal_approx_fast(out=lrec[:], in_=lbc[:])
                    nc.vector.tensor_mul(
                        out=ot[64 * h:64 * (h + 1), c0:c0 + QB],
                        in0=ps_o[h][0:64, :], in1=lrec[:])

            # ---- A2A half (reshard head-space -> row-space) ----
            a2a_outs = {}

            def a2a_half(half):
                a_in = dram.tile([NCORES, 128, RHALF], bf16, tag="a2ain", name="a2ain")
                a_out = dram.tile([NCORES, 128, RHALF], bf16, tag="a2aout", name="a2aout")
                for j in range(NCORES):
                    nc.sync.dma_start(
                        out=a_in[j],
                        in_=ot[:, 2048 * half + RHALF * j:
                               2048 * half + RHALF * (j + 1)])
                nc.gpsimd.collective_compute(
                    "AllToAll", mybir.AluOpType.bypass, replica_groups=rg,
                    ins=[a_in.opt()], outs=[a_out.opt()])
                a2a_outs[half] = a_out

            def proj_half(half):
                a_out = a2a_outs[half]
                pins = []
                for j in range(NCORES):
                    p = pin_pool.tile([128, RHALF], bf16, tag="pin", name="pin")
                    nc.sync.dma_start(out=p[:], in_=a_out[j])
                    pins.append(p)
                for rb in range(RHALF // 128):
                    osb = out_pool.tile([128, D], f32, tag="osb", name="osb")
                    for n in range(2):
                        ps = ps_mm.tile([128, 512], f32, tag="mm", name="mm")
                        for j in range(NCORES):
                            nc.tensor.matmul(
                                ps[:],
                                lhsT=pins[j][:, 128 * rb:128 * (rb + 1)
                                             ],
                                rhs=wproj_sb[:, j, 512 * n:512 * (n + 1)
                                             ],
                                start=(j == 0),
                                stop=(j == NCORES - 1) and not with_bproj)
                        if with_bproj:
                            nc.tensor.matmul(
                                ps[:], lhsT=ones_col[:],
                                rhs=bproj_sb[:, 512 * n:512 * (n + 1)
                                             ],
                                start=False, stop=True)
                        nc.vector.tensor_copy(out=osb[:, 512 * n:512 * (n + 1)],
                                              in_=ps[:])
                    r0 = 256 * half + 128 * rb
                    nc.sync.dma_start(out=out[r0:r0 + 128, :], in_=osb[:])

            # ---- emission order (drives schedule priority) ----
            for tb in range(NTB // 2):
                qkv_tblock(tb)
            v_transpose(0)
            for qb in range(NQB):
                attn_qblock(0, qb)
                qkv_tblock(NTB // 2 + qb)
                v_transpose(1, range(4 * qb, 4 * qb + 4))
            emit_wproj_loads()
            a2a_half(0)
            for qb in range(NQB):
                attn_qblock(1, qb)
            proj_half(0)
            a2a_half(1)
            proj_half(1)

    nc.finalize()
    return nc


def _prep_inputs(x, attention_mask, w_attn, b_attn, w_proj, b_proj):
    x = np.asarray(x, np.float32)
    xt = np.ascontiguousarray(x.reshape(BT, D).T).astype(BF16)
    m2 = (2.0 * np.asarray(attention_mask, np.float32)).reshape(B, T // 128, 128)
    mask2 = np.ascontiguousarray(m2.transpose(2, 0, 1).reshape(128, -1))
    w_attn = np.asarray(w_attn, np.float32)
    b_attn = np.asarray(b_attn, np.float32)
    wp = np.ascontiguousarray(np.asarray(w_proj, np.float32)).astype(BF16)
    scale = 1.0 / np.sqrt(HD)
    with_battn = bool(np.any(b_attn))
    with_bproj = bool(np.any(np.asarray(b_proj)))
    in_maps = []
    for c in range(NCORES):
        h0, h1 = HPC * c, HPC * c + 1
        cols = []
        for h in (h0, h1):
            cols.append(w_attn[:, HD * h:HD * (h + 1)] * scale)        # q
            cols.append(w_attn[:, D + HD * h:D + HD * (h + 1)])        # k
        for h in (h0, h1):
            cols.append(w_attn[:, 2 * D + HD * h:2 * D + HD * (h + 1)])  # v
        wq = np.ascontiguousarray(np.concatenate(cols, axis=1)).astype(BF16)
        m = {"xt": xt, "w_qkv": wq, "w_proj": wp, "mask2": mask2,
             "zeros_in": _ZEROS}
        if with_battn:
            bc = []
            for h in (h0, h1):
                bc.append(b_attn[HD * h:HD * (h + 1)] * scale)
                bc.append(b_attn[D + HD * h:D + HD * (h + 1)])
            for h in (h0, h1):
                bc.append(b_attn[2 * D + HD * h:2 * D + HD * (h + 1)])
            m["b_qkv"] = np.ascontiguousarray(
                np.concatenate(bc)[None, :].astype(BF16))
        if with_bproj:
            m["b_proj"] = np.ascontiguousarray(
                np.asarray(b_proj, np.float32)[None, :].astype(BF16))
        in_maps.append(m)
    return in_maps, with_battn, with_bproj


def _run(inputs, trace=False, tmpdir=None):
    from concourse.bass_utils import run_bass_kernel_spmd

    in_maps, with_battn, with_bproj = _prep_inputs(**inputs)
    key = (with_battn, with_bproj)
    if key not in _CACHE:
        _CACHE[key] = _build(with_battn, with_bproj)
    nc = _CACHE[key]
    res = run_bass_kernel_spmd(nc, in_maps, core_ids=list(range(NCORES)),
                               trace=trace, tmpdir=tmpdir)
    y = np.empty((B, T, D), np.float32)
    for c in range(NCORES):
        o = res.results[c]["out"]
        y[0, RHALF * c:RHALF * (c + 1)] = o[:RHALF]
        y[1, RHALF * c:RHALF * (c + 1)] = o[RHALF:]
    return y, res


def kernel(**inputs) -> np.ndarray:
    y, _ = _run(inputs, trace=False)
    return y


# revision 37
# speedup vs baseline: 1.0139x; 1.0139x over previous
"""Distributed multi-head attention kernel for one TRN2 chip (8 NeuronCores).

Problem: B=2, T=2048, D=1024, H=16 heads (hd=64).
  qkv = x @ w_attn + b_attn ; per-head softmax((q k^T)/sqrt(hd) + 2*mask) v
  out = attn @ w_proj + b_proj

Sharding: tensor-parallel over heads. Core c owns heads {2c, 2c+1}.
  - QKV projection computed in transposed layout (QT/KT: [hd, T]) from
    host-transposed XT, packed so every matmul uses the full 128-wide PE.
  - Attention runs in transposed space: S^T[kv, q] = KT^T-block @ QT-block,
    so the additive mask is a per-partition bias folded into the Exp, and
    the softmax denominator falls out of a ones-column appended to V in the
    PV matmul (O_ext = [V|1]^T @ P^T).
  - Mid-kernel AllToAll (two halves, one per batch) reshards from
    head-space to row-space; the final projection then needs no reduction.
  - All matmuls use float32r (full-rate fp32 PE mode, moving dim 512).
"""

import sys

sys.path.insert(0, "/opt/trn_rl_repo")

import numpy as np

B, T, D = 2, 2048, 1024
H = 16
HD = D // H
NCORES = 8
HPC = H // NCORES          # heads per core = 2
BT = B * T                 # 4096 global rows
ROWS_PER_CORE = BT // NCORES   # 512
RHALF = ROWS_PER_CORE // 2     # 256 rows per A2A half
TB = 512                   # T-block width for QKV projection
NTB = BT // TB             # 8
NKD = D // 128             # 8 contraction chunks over D
QB = 512                   # q-block width in attention
NQB = T // QB              # 4 per (batch, head)
NKV = T // 128             # 16 kv chunks per batch

_CACHE = {}
import ml_dtypes
BF16 = ml_dtypes.bfloat16
_ZEROS = np.zeros((64, 2048), BF16)


def _build(with_battn: bool, with_bproj: bool):
    import concourse.bass as bass
    import concourse.tile as tile
    from concourse import bacc, mybir
    from concourse.masks import make_identity

    f32 = mybir.dt.float32
    bf16 = mybir.dt.bfloat16
    Exp = mybir.ActivationFunctionType.Exp

    nc = bacc.Bacc("TRN2", target_bir_lowering=False, debug=False,
                   num_devices=NCORES)
    rg = [list(range(NCORES))]

    xt = nc.dram_tensor("xt", [D, BT], bf16, kind="ExternalInput")
    w_qkv = nc.dram_tensor("w_qkv", [D, 3 * 128], bf16, kind="ExternalInput")
    w_proj = nc.dram_tensor("w_proj", [D, D], bf16, kind="ExternalInput")
    mask2 = nc.dram_tensor("mask2", [128, B * (T // 128)], f32,
                           kind="ExternalInput")
    zeros_in = nc.dram_tensor("zeros_in", [64, 2048], bf16, kind="ExternalInput")
    if with_battn:
        b_qkv = nc.dram_tensor("b_qkv", [1, 3 * 128], bf16, kind="ExternalInput")
    if with_bproj:
        b_proj = nc.dram_tensor("b_proj", [1, D], bf16, kind="ExternalInput")
    out = nc.dram_tensor("out", [ROWS_PER_CORE, D], f32, kind="ExternalOutput")

    with tile.TileContext(nc, num_cores=NCORES) as tc:
        from contextlib import ExitStack
        with ExitStack() as ctx:
            const = ctx.enter_context(tc.tile_pool(name="const", bufs=1))
            xt_pool = ctx.enter_context(tc.tile_pool(name="xtp", bufs=10))
            qk_pool = ctx.enter_context(tc.tile_pool(name="qkp", bufs=1))
            vnat_pool = ctx.enter_context(tc.tile_pool(name="vnat", bufs=64))
            pt_pool = ctx.enter_context(tc.tile_pool(name="ptp", bufs=5))
            lbc_pool = ctx.enter_context(tc.tile_pool(name="lbc", bufs=2))
            pin_pool = ctx.enter_context(tc.tile_pool(name="pin", bufs=8))
            out_pool = ctx.enter_context(tc.tile_pool(name="outp", bufs=2))
            ps_mm = ctx.enter_context(tc.tile_pool(name="psmm", bufs=4, space="PSUM"))
            ps_ot = ctx.enter_context(tc.tile_pool(name="psot", bufs=4, space="PSUM"))
            dram = ctx.enter_context(tc.tile_pool(name="dram", bufs=2, space="DRAM"))

            # ---- constants ----
            # w_qkv packed per D-chunk: [128, NKD, 384]; group g cols
            # [128g, 128g+128): g0=[q_h0/8|k_h0] g1=[q_h1/8|k_h1] g2=[v_h0|v_h1]
            wqkv_sb = const.tile([128, NKD, 3 * 128], bf16)
            for d in range(NKD):
                nc.sync.dma_start(out=wqkv_sb[:, d, :],
                                  in_=w_qkv[128 * d:128 * (d + 1), :])
            wproj_sb = const.tile([128, NKD, D], bf16)

            def emit_wproj_loads():
                for j in range(NKD):
                    nc.sync.dma_start(out=wproj_sb[:, j, :],
                                      in_=w_proj[128 * j:128 * (j + 1), :])
            # mask (already doubled on host): [128, B, NKV]
            mask_sb = const.tile([128, B, NKV], f32)
            nc.sync.dma_start(out=mask_sb[:],
                              in_=mask2[:].rearrange("p (b j) -> p b j", b=B))
            ident = const.tile([128, 128], bf16)
            make_identity(nc, ident[:])

            if with_battn:
                bqkv_sb = const.tile([1, 3 * 128], bf16)
                nc.sync.dma_start(out=bqkv_sb[:], in_=b_qkv[:])
                ones_row = const.tile([1, TB], bf16)
                nc.vector.memset(ones_row[:], 1.0)
            if with_bproj:
                bproj_sb = const.tile([1, D], bf16)
                nc.sync.dma_start(out=bproj_sb[:], in_=b_proj[:])
                ones_col = const.tile([1, 128], bf16)
                nc.vector.memset(ones_col[:], 1.0)

            # persistent activations. Head h lives at partitions [64h, 64h+64)
            # of both qt2 and kt2, so S^T matmul operands share a base.
            qt2 = qk_pool.tile([128, BT], bf16, tag="qt2", name="qt2")
            # K for head h zero-padded to 128 rows so the S^T matmul streams
            # full-width operands: ktA = [K_h0; 0], ktB = [0; K_h1]
            ktA = qk_pool.tile([128, BT], bf16, tag="ktA", name="ktA")
            ktB = qk_pool.tile([128, BT], bf16, tag="ktB", name="ktB")
            for half in range(2):
                cs = 2048 * half
                nc.sync.dma_start(out=ktA[64:128, cs:cs + 2048],
                                  in_=zeros_in[:])
                nc.sync.dma_start(out=ktB[0:64, cs:cs + 2048],
                                  in_=zeros_in[:])
            vt2 = qk_pool.tile([128, BT], bf16, tag="vt2", name="vt2")
            ot = qk_pool.tile([128, BT], bf16, tag="ot", name="ot")

            # ---- QKV projection (transposed outputs) ----
            def qkv_tblock(tb):
                xts = []
                for d in range(NKD):
                    t = xt_pool.tile([128, TB], bf16, tag="xt", name="xt")
                    nc.sync.dma_start(
                        out=t[:], in_=xt[128 * d:128 * (d + 1),
                                         TB * tb:TB * (tb + 1)])
                    xts.append(t)
                for g in range(3):
                    ps = ps_mm.tile([128, TB], f32, tag="mm", name="mm")
                    for d in range(NKD):
                        nc.tensor.matmul(
                            ps[:],
                            lhsT=wqkv_sb[:, d, 128 * g:128 * (g + 1)],
                            rhs=xts[d][:],
                            start=(d == 0),
                            stop=(d == NKD - 1) and not with_battn)
                    if with_battn:
                        nc.tensor.matmul(
                            ps[:],
                            lhsT=bqkv_sb[:, 128 * g:128 * (g + 1)],
                            rhs=ones_row[:],
                            start=False, stop=True)
                    if g < 2:
                        nc.scalar.copy(
                            out=qt2[64 * g:64 * (g + 1), TB * tb:TB * (tb + 1)],
                            in_=ps[0:64, :])
                        ktdst = ktA if g == 0 else ktB
                        nc.scalar.copy(
                            out=ktdst[64 * g:64 * (g + 1), TB * tb:TB * (tb + 1)],
                            in_=ps[64:128, :])
                    else:
                        nc.vector.tensor_copy(
                            out=vt2[:, TB * tb:TB * (tb + 1)], in_=ps[:])

            # ---- V transpose to natural layout (+ ones col for denom) ----
            vnat = {}

            def v_transpose(b, js=None):
                for j in (range(NKV) if js is None else js):
                    pst = ps_mm.tile([128, 128], bf16, tag="mm", name="tr",
                                     padded_shape=[128, 512])
                    nc.tensor.transpose(
                        pst[:], vt2[:, 2048 * b + 128 * j: 2048 * b + 128 * (j + 1)],
                        ident[:])
                    for h in range(HPC):
                        vtile = vnat_pool.tile([128, 128], bf16, tag="vnat", name="vnat")
                        nc.vector.tensor_copy(out=vtile[:, 0:64],
                                              in_=pst[:, 64 * h:64 * (h + 1)])
                        nc.vector.memset(vtile[:, 64:65], 1.0)
                        vnat[(b, h, j)] = vtile

            # ---- attention for one (batch, q-block), both heads ----
            # Head h operands live at partition base 64h, so the two S^T
            # matmuls land on disjoint PE row-groups (tile_position auto
            # (0,0)/(64,0)) and execute concurrently.
            def attn_qblock(b, qb):
                c0 = 2048 * b + QB * qb
                # stage the q-block contiguously: the S^T moving operand
                # streams ~20% faster from a dense tile than a strided slice
                qsl = pt_pool.tile([128, QB], bf16, tag="qst", name="qst")
                nc.vector.tensor_copy(out=qsl[:], in_=qt2[:, c0:c0 + QB])
                qsl = qsl[:]
                ps_o = [ps_ot.tile([128, QB], f32, tag="ot", name="ot")
                        for _ in range(HPC)]
                for j in range(NKV):
                    k0 = 2048 * b + 128 * j
                    pts = []
                    for h in range(HPC):
                        ps_s = ps_mm.tile([128, QB], f32, tag="mm", name="mm")
                        nc.tensor.matmul(
                            ps_s[:],
                            lhsT=(ktA if h == 0 else ktB)[:, k0:k0 + 128],
                            rhs=qsl,
                            start=True, stop=True)
                        pt = pt_pool.tile([128, QB], bf16, tag="pt", name="pt")
                        nc.scalar.activation(out=pt[:], in_=ps_s[:], func=Exp,
                                             bias=mask_sb[:, b, j:j + 1],
                                             scale=1.0)
                        pts.append(pt)
                    for h in range(HPC):
                        nc.tensor.matmul(
                            ps_o[h][:],
                            lhsT=vnat[(b, h, j)][:],  # [V_h|1|pad] full width
                            rhs=pts[h][:],
                            start=(j == 0), stop=(j == NKV - 1),
                            skip_group_check=True)
                # normalize by softmax denom (row 64 of ps_o) and write OT
                for h in range(HPC):
                    lsb = lbc_pool.tile([1, QB], f32, tag="lsb", name="lsb")
                    nc.vector.tensor_copy(out=lsb[:], in_=ps_o[h][64:65, :])
                    ldram = dram.tile([1, QB], f32, tag="ld", name="ld")
                    nc.sync.dma_start(out=ldram[:], in_=lsb[:])
                    lbc = lbc_pool.tile([64, QB], f32, tag="lbc", name="lbc")
                    nc.sync.dma_start(out=lbc[:],
                                      in_=ldram[:].to_broadcast([64, QB]))
                    lrec = lbc_pool.tile([64, QB], f32, tag="lrec", name="lrec")
                    nc.vector.reciprocal_approx_fast(out=lrec[:], in_=lbc[:])
                    nc.vector.tensor_mul(
                        out=ot[64 * h:64 * (h + 1), c0:c0 + QB],
                        in0=ps_o[h][0:64, :], in1=lrec[:])

            # ---- A2A half (reshard head-space -> row-space) ----
            a2a_outs = {}

            def a2a_half(half):
                a_in = dram.tile([NCORES, 128, RHALF], bf16, tag="a2ain", name="a2ain")
                a_out = dram.tile([NCORES, 128, RHALF], bf16, tag="a2aout", name="a2aout")
                for j in range(NCORES):
                    nc.sync.dma_start(
                        out=a_in[j],
                        in_=ot[:, 2048 * half + RHALF * j:
                               2048 * half + RHALF * (j + 1)])
                nc.gpsimd.collective_compute(
                    "AllToAll", mybir.AluOpType.bypass, replica_groups=rg,
                    ins=[a_in.opt()], outs=[a_out.opt()])
                a2a_outs[half] = a_out

            def proj_half(half):
                a_out = a2a_outs[half]
                pins = []
                for j in range(NCORES):
                    p = pin_pool.tile([128, RHALF], bf16, tag="pin", name="pin")
                    nc.sync.dma_start(out=p[:], in_=a_out[j])
                    pins.append(p)
                for rb in range(RHALF // 128):
                    osb = out_pool.tile([128, D], f32, tag="osb", name="osb")
                    for n in range(2):
                        ps = ps_mm.tile([128, 512], f32, tag="mm", name="mm")
                        for j in range(NCORES):
                            nc.tensor.matmul(
                                ps[:],
                                lhsT=pins[j][:, 128 * rb:128 * (rb + 1)
                                             ],
                                rhs=wproj_sb[:, j, 512 * n:512 * (n + 1)
                                             ],
                                start=(j == 0),
                                stop=(j == NCORES - 1) and not with_bproj)
                        if with_bproj:
                            nc.tensor.matmul(
                                ps[:], lhsT=ones_col[:],
                                rhs=bproj_sb[:, 512 * n:512 * (n + 1)
                                             ],
                                start=False, stop=True)
                        nc.vector.tensor_copy(out=osb[:, 512 * n:512 * (n + 1)],
                                              in_=ps[:])
                    r0 = 256 * half + 128 * rb
                    nc.sync.dma_start(out=out[r0:r0 + 128, :], in_=osb[:])

            # ---- emission order (drives schedule priority) ----
            for tb in range(NTB // 2):
                qkv_tblock(tb)
            v_transpose(0)
            for qb in range(NQB):
                attn_qblock(0, qb)
                qkv_tblock(NTB // 2 + qb)
            v_transpose(1)
            emit_wproj_loads()
            a2a_half(0)
            for qb in range(NQB):
                attn_qblock(1, qb)
            proj_half(0)
            a2a_half(1)
            proj_half(1)

    nc.finalize()
    return nc


def _prep_inputs(x, attention_mask, w_attn, b_attn, w_proj, b_proj):
    x = np.asarray(x, np.float32)
    xt = np.ascontiguousarray(x.reshape(BT, D).T).astype(BF16)
    m2 = (2.0 * np.asarray(attention_mask, np.float32)).reshape(B, T // 128, 128)
    mask2 = np.ascontiguousarray(m2.transpose(2, 0, 1).reshape(128, -1))
    w_attn = np.asarray(w_attn, np.float32)
    b_attn = np.asarray(b_attn, np.float32)
    wp = np.ascontiguousarray(np.asarray(w_proj, np.float32)).astype(BF16)
    scale = 1.0 / np.sqrt(HD)
    with_battn = bool(np.any(b_attn))
    with_bproj = bool(np.any(np.asarray(b_proj)))
    in_maps = []
    for c in range(NCORES):
        h0, h1 = HPC * c, HPC * c + 1
        cols = []
        for h in (h0, h1):
            cols.append(w_attn[:, HD * h:HD * (h + 1)] * scale)        # q
            cols.append(w_attn[:, D + HD * h:D + HD * (h + 1)])        # k
        for h in (h0, h1):
            cols.append(w_attn[:, 2 * D + HD * h:2 * D + HD * (h + 1)])  # v
        wq = np.ascontiguousarray(np.concatenate(cols, axis=1)).astype(BF16)
        m = {"xt": xt, "w_qkv": wq, "w_proj": wp, "mask2": mask2,
             "zeros_in": _ZEROS}
        if with_battn:
            bc = []
            for h in (h0, h1):
                bc.append(b_attn[HD * h:HD * (h + 1)] * scale)
                bc.append(b_attn[D + HD * h:D + HD * (h + 1)])
            for h in (h0, h1):
                bc.append(b_attn[2 * D + HD * h:2 * D + HD * (h + 1)])
            m["b_qkv"] = np.ascontiguousarray(
                np.concatenate(bc)[None, :].astype(BF16))
        if with_bproj:
            m["b_proj"] = np.ascontiguousarray(
                np.asarray(b_proj, np.float32)[None, :].astype(BF16))
        in_maps.append(m)
    return in_maps, with_battn, with_bproj


def _run(inputs, trace=False, tmpdir=None):
    from concourse.bass_utils import run_bass_kernel_spmd

    in_maps, with_battn, with_bproj = _prep_inputs(**inputs)
    key = (with_battn, with_bproj)
    if key not in _CACHE:
        _CACHE[key] = _build(with_battn, with_bproj)
    nc = _CACHE[key]
    res = run_bass_kernel_spmd(nc, in_maps, core_ids=list(range(NCORES)),
                               trace=trace, tmpdir=tmpdir)
    y = np.empty((B, T, D), np.float32)
    for c in range(NCORES):
        o = res.results[c]["out"]
        y[0, RHALF * c:RHALF * (c + 1)] = o[:RHALF]
        y[1, RHALF * c:RHALF * (c + 1)] = o[RHALF:]
    return y, res


def kernel(**inputs) -> np.ndarray:
    y, _ = _run(inputs, trace=False)
    return y
